# revision 6
# baseline (speedup 1.0000x reference)
"""GPT forward on 8 Trainium2 NeuronCores (Bass/Tile).

Sharding: DP=2 over batch (cores 0-3 batch 0, cores 4-7 batch 1).
Within each 4-core group, tokens are sharded 4 ways with a low+high
128-row block pair per core so causal attention work is balanced:
  rank c owns q rows [128c, 128c+128) u [1024-128(c+1), 1024-128c).
Activations are feature-major (feature partition, token free); fp32
residual stream, bf16 matmul operands. One AllGather for K and one for
V (bf16) per layer within the 4-core group. Causality is enforced by
per-core 0/1 masks multiplied into exp(scores) — the device program is
identical on every core (SPMD); only the input data differs.
LM head is token-parallel: every core computes its 256 tokens x full
(padded) vocab.
"""
import os
import numpy as np
import ml_dtypes

import concourse.bass as bass
import concourse.mybir as mybir
import concourse.tile as tile
from concourse import bacc
from concourse.bass_utils import run_bass_kernel_spmd

F32 = mybir.dt.float32
F32R = mybir.dt.float32r
BF16 = mybir.dt.bfloat16
AF = mybir.ActivationFunctionType
ALU = mybir.AluOpType

B, T, D, H, V, L = 2, 1024, 1024, 16, 32000, 8
DH = D // H            # 64
DFF = 4 * D            # 4096
EPS = 1e-5
NCORE = 8
G = 4                  # cores per batch group
TLOC = 256             # tokens per core (two 128-blocks)
KT = D // 128          # 8 feature tiles
VPAD = 32256           # 252 * 128
NL = int(os.environ.get("GPT_NLAYERS", str(L)))

bfdt = ml_dtypes.bfloat16


def _q_starts(c):
    return 128 * c, 1024 - 128 * (c + 1)


def _kv_start(j):
    """Global row start of gathered kv 128-block j (= 2r+s)."""
    r, s = j // 2, j % 2
    a, b = _q_starts(r)
    return a if s == 0 else b


# ---------------------------------------------------------------------------
# device kernel
# ---------------------------------------------------------------------------

def build_nc():
    nc = bacc.Bacc(None, target_bir_lowering=False, num_devices=NCORE)
    P = {}

    def par(name, shape, dtype, out=False):
        P[name] = nc.declare_dram_parameter(name, list(shape), dtype,
                                            isOutput=out)

    par("x0", (KT, 128, TLOC), F32)
    par("amask", (4, 128, 512), BF16)       # per-core causal masks, block pairs
    for l in range(NL):
        par(f"wqkv{l}", (KT, 6, 128, 512), BF16)
        par(f"wo{l}", (KT, 2, 128, 512), BF16)
        par(f"w1{l}", (KT, 8, 128, 512), BF16)
        par(f"w2{l}", (DFF // 128, 2, 128, 512), BF16)
        par(f"ln{l}", (128, 32), F32)       # [ln1_s|ln1_b|ln2_s|ln2_b] x 8 cols
        par(f"bqkv{l}", (128, 24), F32)     # [bq|bk|bv] feature-tiled
        par(f"bvrow{l}", (1, D), F32)       # bv in row layout
        par(f"bo{l}", (128, 8), F32)
        par(f"b1{l}", (128, 32), F32)
        par(f"b2{l}", (128, 8), F32)
    par("lnf", (128, 16), F32)
    par("wl", (KT, VPAD // 512, 128, 512), BF16)
    par("bl", (128, VPAD // 128), F32)
    par("logits", (VPAD // 128, 128, TLOC), F32, out=True)

    with tile.TileContext(nc) as tc:
        _body(tc, nc, P)
    nc.compile()
    return nc


def _body(tc, nc, P):
    from contextlib import ExitStack
    ctx = ExitStack()
    sb = ctx.enter_context(tc.tile_pool(name="sb", bufs=1))
    wp = ctx.enter_context(tc.tile_pool(name="wp", bufs=1))
    pp = ctx.enter_context(tc.tile_pool(name="pp", bufs=1))
    dram = ctx.enter_context(tc.tile_pool(name="dram", bufs=1, space="DRAM"))
    ps = ctx.enter_context(tc.tile_pool(name="psum", bufs=1, space="PSUM"))

    def st(pool, shape, dtype, tag, bufs=1, **kw):
        return pool.tile(shape, dtype, tag=tag, bufs=bufs, name=tag, **kw)

    # persistent state
    x = [st(sb, [128, TLOC], F32, f"x{f}") for f in range(KT)]
    xh = [st(sb, [128, TLOC], BF16, f"xh{f}") for f in range(KT)]
    q_sb = [st(sb, [128, TLOC], BF16, f"q{f}") for f in range(KT)]
    kloc = [st(sb, [128, TLOC], BF16, f"kl{f}") for f in range(KT)]
    vloc = [st(sb, [128, D], BF16, f"vl{t}") for t in range(2)]
    ksb = [st(sb, [128, G * TLOC], BF16, f"kg{f}") for f in range(KT)]
    vsb = [st(sb, [128, 16 * 65], BF16, f"vg{j}") for j in range(8)]
    y_sb = [st(sb, [128, TLOC], BF16, f"y{f}") for f in range(KT)]
    gt = [st(sb, [128, TLOC], BF16, f"g{m}") for m in range(DFF // 128)]
    ones_b = st(sb, [128, 1], BF16, "ones_b")
    nc.vector.memset(ones_b[:], 1.0)
    amask = st(sb, [128, 4 * 512], BF16, "amask")
    nc.sync.dma_start(out=amask[:].rearrange("p (j q) -> p j q", j=4),
                      in_=P["amask"][:].transpose([1, 0, 2]))

    def psx():
        return [st(ps, [128, TLOC], F32, t, bufs=b)
                for (t, b) in (("ps_x", 2), ("ps_x", 2),
                               ("ps_o", 2), ("ps_s", 3))]

    def load_w(pname, g, k):
        wt = st(wp, [128, 512], BF16, "w", bufs=6)
        nc.sync.dma_start(out=wt[:], in_=P[pname][k, g])
        return wt

    def wproj(pname, mg, nk, rhs, pss):
        """pss[m] += sum_k W[k, mg, :, 128m:128m+128].T @ rhs[k]"""
        for k in range(nk):
            wt = load_w(pname, mg, k)
            for m in range(4):
                nc.tensor.matmul(pss[m][:], wt[:, 128 * m:128 * (m + 1)],
                                 rhs[k][:], start=(k == 0), stop=(k == nk - 1))

    def ln_layer(scol, bcol, dst):
        s_ps = st(ps, [1, TLOC], F32, "ps_s", bufs=3)
        s2_ps = st(ps, [1, TLOC], F32, "ps_s", bufs=3)
        for f in range(KT):
            xb = st(pp, [128, TLOC], BF16, "xb", bufs=2)
            xsq = st(pp, [128, TLOC], BF16, "xsq", bufs=2)
            nc.vector.tensor_copy(xb[:], x[f][:])
            nc.scalar.square(xsq[:], x[f][:])
            nc.tensor.matmul(s_ps[:], ones_b[:], xb[:],
                             start=(f == 0), stop=(f == KT - 1))
            nc.tensor.matmul(s2_ps[:], ones_b[:], xsq[:],
                             start=(f == 0), stop=(f == KT - 1))
        mean = st(pp, [1, TLOC], F32, "stat", bufs=8)
        msq = st(pp, [1, TLOC], F32, "stat", bufs=8)
        mm = st(pp, [1, TLOC], F32, "stat", bufs=8)
        var = st(pp, [1, TLOC], F32, "stat", bufs=8)
        rinv = st(pp, [1, TLOC], F32, "stat", bufs=8)
        rstd = st(pp, [1, TLOC], F32, "stat", bufs=8)
        nc.vector.tensor_scalar_mul(mean[:], s_ps[:], 1.0 / D)
        nc.vector.tensor_scalar_mul(msq[:], s2_ps[:], 1.0 / D)
        nc.vector.tensor_mul(mm[:], mean[:], mean[:])
        nc.vector.scalar_tensor_tensor(var[:], msq[:], float(EPS), mm[:],
                                       ALU.add, ALU.subtract)
        nc.vector.reciprocal(rinv[:], var[:])
        nc.scalar.sqrt(rstd[:], rinv[:])
        m_bc = st(pp, [128, TLOC], F32, "mbc", bufs=2)
        r_bc = st(pp, [128, TLOC], F32, "rbc", bufs=2)
        nc.gpsimd.partition_broadcast(m_bc[:], mean[:])
        nc.gpsimd.partition_broadcast(r_bc[:], rstd[:])
        for f in range(KT):
            t0 = st(pp, [128, TLOC], F32, "lnt0", bufs=2)
            t1 = st(pp, [128, TLOC], F32, "lnt1", bufs=2)
            nc.vector.tensor_sub(t0[:], x[f][:], m_bc[:])
            nc.vector.tensor_mul(t1[:], t0[:], r_bc[:])
            nc.gpsimd.tensor_scalar(dst[f][:], t1[:], scol[f], bcol[f],
                                    ALU.mult, ALU.add)

    for f in range(KT):
        nc.sync.dma_start(out=x[f][:], in_=P["x0"][f])

    group = [list(range(G)), list(range(G, 2 * G))]

    for l in range(NL):
        lnp = st(pp, [128, 32], F32, "lnp", bufs=2)
        nc.sync.dma_start(out=lnp[:], in_=P[f"ln{l}"][:])
        bqkv = st(pp, [128, 24], F32, "bqkvt", bufs=2)
        nc.sync.dma_start(out=bqkv[:], in_=P[f"bqkv{l}"][:])
        bvrow = st(pp, [1, D], F32, "bvrow", bufs=2)
        nc.sync.dma_start(out=bvrow[:], in_=P[f"bvrow{l}"][:])
        bo = st(pp, [128, 8], F32, "bot", bufs=2)
        nc.sync.dma_start(out=bo[:], in_=P[f"bo{l}"][:])
        b1 = st(pp, [128, 32], F32, "b1t", bufs=2)
        nc.sync.dma_start(out=b1[:], in_=P[f"b1{l}"][:])
        b2 = st(pp, [128, 8], F32, "b2t", bufs=2)
        nc.sync.dma_start(out=b2[:], in_=P[f"b2{l}"][:])

        # LN1 -> xh
        ln_layer([lnp[:, f:f + 1] for f in range(KT)],
                 [lnp[:, 8 + f:9 + f] for f in range(KT)], xh)

        # Q, K projections (feature-major)
        for proj, dst, boff in ((0, q_sb, 0), (1, kloc, 8)):
            for mg in range(2):
                pss = psx()
                wproj(f"wqkv{l}", 2 * proj + mg, KT, xh, pss)
                for m in range(4):
                    fm = 4 * mg + m
                    nc.vector.tensor_scalar_add(
                        dst[fm][:], pss[m][:],
                        bqkv[:, boff + fm:boff + fm + 1])

        # V projection (token-major): vloc[th][:, 512n:...] = xh_th.T @ Wv_n
        bv_bc = st(pp, [128, D], F32, "bvbc", bufs=2)
        nc.gpsimd.partition_broadcast(bv_bc[:], bvrow[:])
        for th in range(2):
            for n in range(2):
                psv = st(ps, [128, 512], F32, "ps_s", bufs=3)
                for k in range(KT):
                    wt = load_w(f"wqkv{l}", 4 + n, k)
                    nc.tensor.matmul(psv[:],
                                     xh[k][:, 128 * th:128 * (th + 1)],
                                     wt[:],
                                     start=(k == 0), stop=(k == KT - 1))
                nc.vector.tensor_add(vloc[th][:, 512 * n:512 * (n + 1)],
                                     psv[:], bv_bc[:, 512 * n:512 * (n + 1)])

        # stage K/V to DRAM and AllGather within the batch group
        k_in = st(dram, [D, TLOC], BF16, "k_in", bufs=2)
        k_out = st(dram, [G * D, TLOC], BF16, "k_out", bufs=2)
        v_in = st(dram, [2 * 128, D], BF16, "v_in", bufs=2)
        v_out = st(dram, [G * 2 * 128, D], BF16, "v_out", bufs=2)
        for f in range(KT):
            nc.gpsimd.dma_start(out=k_in[128 * f:128 * (f + 1), :],
                                in_=kloc[f][:])
        for th in range(2):
            nc.gpsimd.dma_start(out=v_in[128 * th:128 * (th + 1), :],
                                in_=vloc[th][:])
        nc.gpsimd.collective_compute(
            "AllGather", ALU.bypass, replica_groups=group,
            ins=[k_in[:].opt()], outs=[k_out[:].opt()])
        nc.gpsimd.collective_compute(
            "AllGather", ALU.bypass, replica_groups=group,
            ins=[v_in[:].opt()], outs=[v_out[:].opt()])
        for f in range(KT):
            src = k_out[:].rearrange("(g p q) t -> p q g t", g=G, p=KT)[f]
            nc.sync.dma_start(
                out=ksb[f][:].rearrange("p (g t) -> p g t", g=G), in_=src)
        for j in range(8):
            src = v_out[128 * j:128 * (j + 1), :]
            nc.sync.dma_start(
                out=vsb[j][:].rearrange("p (h e) -> p h e", h=16)[:, :, 0:64],
                in_=src.rearrange("p (h e) -> p h e", h=16))
            nc.vector.memset(
                vsb[j][:].rearrange("p (h e) -> p h e", h=16)[:, :, 64:65],
                1.0)

        # attention: all 8 kv blocks (block pairs share a PSUM bank), masks
        for h in range(H):
            f, po = h // 2, 64 * (h % 2)
            o_ps = st(ps, [65, TLOC], F32, "ps_o", bufs=2)
            for jp in range(4):
                s_ps = st(ps, [128, 512], F32, "ps_s", bufs=3)
                for s in range(2):
                    j = 2 * jp + s
                    nc.tensor.matmul(
                        s_ps[:, 256 * s:256 * (s + 1)],
                        ksb[f][po:po + 64, 128 * j:128 * (j + 1)],
                        q_sb[f][po:po + 64, :],
                        start=True, stop=True)
                pt = st(pp, [128, 512], BF16, "pt", bufs=6)
                nc.scalar.activation(pt[:], s_ps[:], AF.Exp,
                                     scale=float(1.0 / np.sqrt(DH)))
                nc.vector.tensor_mul(pt[:], pt[:],
                                     amask[:, 512 * jp:512 * (jp + 1)])
                for s in range(2):
                    j = 2 * jp + s
                    nc.tensor.matmul(
                        o_ps[:],
                        vsb[j][:, 65 * h:65 * (h + 1)],
                        pt[:, 256 * s:256 * (s + 1)],
                        start=(jp == 0 and s == 0), stop=(jp == 3 and s == 1),
                        skip_group_check=True)
            dinv = st(pp, [1, TLOC], F32, "dinv", bufs=3)
            nc.vector.reciprocal(dinv[:], o_ps[64:65, :])
            dbc = st(pp, [64, TLOC], F32, "dbc", bufs=3)
            nc.gpsimd.partition_broadcast(dbc[:], dinv[:])
            nc.vector.tensor_mul(y_sb[f][po:po + 64, :], o_ps[0:64, :],
                                 dbc[:])

        # Wo + residual
        for mg in range(2):
            pss = psx()
            wproj(f"wo{l}", mg, KT, y_sb, pss)
            for m in range(4):
                fm = 4 * mg + m
                nc.vector.scalar_tensor_tensor(
                    x[fm][:], pss[m][:], bo[:, fm:fm + 1], x[fm][:],
                    ALU.add, ALU.add)

        # LN2 -> xh
        ln_layer([lnp[:, 16 + f:17 + f] for f in range(KT)],
                 [lnp[:, 24 + f:25 + f] for f in range(KT)], xh)

        # FFN
        for mg in range(8):
            pss = psx()
            wproj(f"w1{l}", mg, KT, xh, pss)
            for m in range(4):
                fm = 4 * mg + m
                nc.scalar.activation(gt[fm][:], pss[m][:], AF.Gelu,
                                     bias=b1[:, fm:fm + 1])
        for mg in range(2):
            pss = psx()
            wproj(f"w2{l}", mg, DFF // 128, gt, pss)
            for m in range(4):
                fm = 4 * mg + m
                nc.vector.scalar_tensor_tensor(
                    x[fm][:], pss[m][:], b2[:, fm:fm + 1], x[fm][:],
                    ALU.add, ALU.add)

    # final LN + LM head
    lnf = st(pp, [128, 16], F32, "lnft", bufs=1)
    nc.sync.dma_start(out=lnf[:], in_=P["lnf"][:])
    ln_layer([lnf[:, f:f + 1] for f in range(KT)],
             [lnf[:, 8 + f:9 + f] for f in range(KT)], xh)
    bl_sb = st(sb, [128, VPAD // 128], F32, "bl")
    nc.sync.dma_start(out=bl_sb[:], in_=P["bl"][:])
    for mg in range(VPAD // 512):
        pss = psx()
        wproj("wl", mg, KT, xh, pss)
        for m in range(4):
            vm = 4 * mg + m
            osb = st(pp, [128, TLOC], F32, "lout", bufs=4)
            nc.vector.tensor_scalar_add(osb[:], pss[m][:],
                                        bl_sb[:, vm:vm + 1])
            nc.sync.dma_start(out=P["logits"][vm], in_=osb[:])

    ctx.close()


# ---------------------------------------------------------------------------
# host side
# ---------------------------------------------------------------------------

_CACHE = {}


def _tile_w(w):
    din, dout = w.shape
    t = w.reshape(din // 128, 128, dout // 512, 512).transpose(0, 2, 1, 3)
    return np.ascontiguousarray(t).astype(bfdt)


def _feat_cols(v):
    return np.ascontiguousarray(np.asarray(v, np.float32).reshape(-1, 128).T)


def _core_rows(c):
    a, b = _q_starts(c)
    return list(range(a, a + 128)) + list(range(b, b + 128))


def _core_mask(c):
    """(4, 128, 512) bf16: mask[jp][j, 256s+i] = kv_glob <= q_glob."""
    qrows = np.array(_core_rows(c))                     # (256,)
    m = np.zeros((4, 128, 512), np.float32)
    for jp in range(4):
        for s in range(2):
            j = 2 * jp + s
            kvrows = _kv_start(j) + np.arange(128)      # (128,)
            m[jp][:, 256 * s:256 * (s + 1)] = (
                kvrows[:, None] <= qrows[None, :])
    return m.astype(bfdt)


def _get_nc():
    if "nc" not in _CACHE:
        _CACHE["nc"] = build_nc()
    return _CACHE["nc"]


def kernel(tokens, emb, pos_emb, ln1_s, ln1_b, Wq, bq, Wk, bk, Wv, bv, Wo, bo,
           ln2_s, ln2_b, W1, b1, W2, b2, lnf_s, lnf_b, Wl, bl):
    f32 = lambda a: np.asarray(a, np.float32)
    tokens = np.asarray(tokens)
    x0 = f32(emb)[tokens] + f32(pos_emb)[:T][None]      # (B, T, D)

    shared = {}
    for l in range(NL):
        wqkv = np.concatenate([f32(Wq)[l], f32(Wk)[l], f32(Wv)[l]], axis=1)
        shared[f"wqkv{l}"] = _tile_w(wqkv)
        shared[f"wo{l}"] = _tile_w(f32(Wo)[l])
        shared[f"w1{l}"] = _tile_w(f32(W1)[l])
        shared[f"w2{l}"] = _tile_w(f32(W2)[l])
        shared[f"ln{l}"] = np.concatenate(
            [_feat_cols(ln1_s[l]), _feat_cols(ln1_b[l]),
             _feat_cols(ln2_s[l]), _feat_cols(ln2_b[l])], axis=1)
        shared[f"bqkv{l}"] = np.concatenate(
            [_feat_cols(bq[l]), _feat_cols(bk[l]), _feat_cols(bv[l])], axis=1)
        shared[f"bvrow{l}"] = f32(bv)[l].reshape(1, D).copy()
        shared[f"bo{l}"] = _feat_cols(bo[l])
        shared[f"b1{l}"] = _feat_cols(b1[l])
        shared[f"b2{l}"] = _feat_cols(b2[l])
    shared["lnf"] = np.concatenate(
        [_feat_cols(lnf_s), _feat_cols(lnf_b)], axis=1)
    wl_pad = np.zeros((D, VPAD), np.float32)
    wl_pad[:, :V] = f32(Wl)
    shared["wl"] = _tile_w(wl_pad)
    bl_pad = np.zeros(VPAD, np.float32)
    bl_pad[:V] = f32(bl)
    shared["bl"] = _feat_cols(bl_pad)

    in_maps = []
    for core in range(NCORE):
        g, c = core // G, core % G
        m = dict(shared)
        xt = x0[g][_core_rows(c)].T                     # (D, 256)
        m["x0"] = np.ascontiguousarray(xt).reshape(KT, 128, TLOC).copy()
        m["amask"] = _core_mask(c)
        in_maps.append(m)

    res = run_bass_kernel_spmd(_get_nc(), in_maps, list(range(NCORE)))

    global LAST_EXEC_NS
    LAST_EXEC_NS = getattr(res, "exec_time_ns", None)

    out = np.empty((B, T, V), np.float32)
    for core in range(NCORE):
        g, c = core // G, core % G
        lg = res.results[core]["logits"].reshape(VPAD, TLOC)[:V]
        out[g, _core_rows(c), :] = lg.T
    return out


LAST_EXEC_NS = None


# revision 16
# speedup vs baseline: 2.5743x; 2.5743x over previous
"""GPT forward on 8 Trainium2 NeuronCores (Bass/Tile).

Sharding: DP=2 over batch (cores 0-3 batch 0, cores 4-7 batch 1).
Within each 4-core group, tokens are sharded 4 ways with a low+high
128-row block pair per core so causal attention work is balanced:
  rank c owns q rows [128c, 128c+128) u [1024-128(c+1), 1024-128c).
Activations are feature-major (feature partition, token free); fp32
residual stream, bf16 matmul operands. One fused K+V AllGather (bf16)
per layer within the 4-core group. Causality is enforced by per-core
0/1 masks multiplied into exp(scores) — the device program is
identical on every core (SPMD); only the input data differs. Local q
columns 0:128 (the low block) can never see the high kv blocks, and
columns 128:256 (the high block) always fully see the low kv blocks,
so those cases are resolved statically. LM head is token-parallel:
every core computes its 256 tokens x the full (padded) vocab.
"""
import os
import numpy as np
import ml_dtypes

import concourse.bass as bass
import concourse.mybir as mybir
import concourse.tile as tile
from concourse import bacc
from concourse.bass_utils import run_bass_kernel_spmd

F32 = mybir.dt.float32
BF16 = mybir.dt.bfloat16
AF = mybir.ActivationFunctionType
ALU = mybir.AluOpType

B, T, D, H, V, L = 2, 1024, 1024, 16, 32000, 8
DH = D // H            # 64
DFF = 4 * D            # 4096
EPS = 1e-5
NCORE = 8
G = 4                  # cores per batch group
TLOC = 256             # tokens per core (two 128-blocks)
KT = D // 128          # 8 feature tiles
VPAD = 32256           # 252 * 128
NL = int(os.environ.get("GPT_NLAYERS", str(L)))
NOAG = os.environ.get("GPT_NOAG", "0") == "1"

bfdt = ml_dtypes.bfloat16


def _q_starts(c):
    return 128 * c, 1024 - 128 * (c + 1)


def _kv_start(j):
    """Global row start of gathered kv 128-block j (= 2r+s)."""
    r, s = j // 2, j % 2
    a, b = _q_starts(r)
    return a if s == 0 else b


A_BLOCKS = [0, 2, 4, 6]   # gathered idx of the four low (A) kv blocks
B_BLOCKS = [1, 3, 5, 7]   # the four high (B) kv blocks


# ---------------------------------------------------------------------------
# device kernel
# ---------------------------------------------------------------------------

def build_nc():
    nc = bacc.Bacc(None, target_bir_lowering=False, num_devices=NCORE)
    P = {}

    def par(name, shape, dtype, out=False):
        P[name] = nc.declare_dram_parameter(name, list(shape), dtype,
                                            isOutput=out)

    par("x0", (KT, 128, TLOC), F32)
    par("maska", (2, 128, 256), BF16)   # pair jp: [blk(2) x 128] for q 0:128
    par("maskb", (128, 512), BF16)      # [blk(4) x 128] for q cols 128:256
    for l in range(NL):
        par(f"wqkv{l}", (6, 128, KT * 512), BF16)
        par(f"wo{l}", (2, 128, KT * 512), BF16)
        par(f"w1{l}", (8, 128, KT * 512), BF16)
        par(f"w2{l}", (2, 128, (DFF // 128) * 512), BF16)
        par(f"ln{l}", (128, 32), F32)   # [ln1_s|ln1_b|ln2_s|ln2_b] x 8
        par(f"bqkv{l}", (128, 24), F32)
        par(f"bvrow{l}", (1, D), F32)
        par(f"bo{l}", (128, 8), F32)
        par(f"b1{l}", (128, 32), F32)
        par(f"b2{l}", (128, 8), F32)
    par("lnf", (128, 16), F32)
    par("wl", (VPAD // 512, 128, KT * 512), BF16)
    par("bl", (128, VPAD // 128), F32)
    par("logits", (VPAD // 512, 128, 1024), F32, out=True)

    with tile.TileContext(nc) as tc:
        _body(tc, nc, P)
    nc.compile()
    return nc


def _body(tc, nc, P):
    from contextlib import ExitStack
    ctx = ExitStack()
    sb = ctx.enter_context(tc.tile_pool(name="sb", bufs=1))
    wp = ctx.enter_context(tc.tile_pool(name="wp", bufs=1))
    pp = ctx.enter_context(tc.tile_pool(name="pp", bufs=1))
    dram = ctx.enter_context(tc.tile_pool(name="dram", bufs=1, space="DRAM"))
    ps = ctx.enter_context(tc.tile_pool(name="psum", bufs=1, space="PSUM"))

    def st(pool, shape, dtype, tag, bufs=1, **kw):
        return pool.tile(shape, dtype, tag=tag, bufs=bufs, name=tag, **kw)

    dma_engs = [nc.sync, nc.scalar]
    dma_i = [0]

    def dma(out, in_):
        e = dma_engs[dma_i[0] % len(dma_engs)]
        dma_i[0] += 1
        e.dma_start(out=out, in_=in_)

    # persistent state
    x = [st(sb, [128, TLOC], F32, f"x{f}") for f in range(KT)]
    xh = [st(sb, [128, TLOC], BF16, f"xh{f}") for f in range(KT)]
    q_sb = [st(sb, [128, TLOC], BF16, f"q{f}") for f in range(KT)]
    xhg = [st(sb, [128, G * TLOC], BF16, f"xg{f}") for f in range(KT)]
    ksb = [st(sb, [128, G * TLOC], BF16, f"kg{f}") for f in range(KT)]
    vsb = [st(sb, [128, 16 * 65], BF16, f"vg{j}") for j in range(8)]
    y_sb = [st(sb, [128, TLOC], BF16, f"y{f}") for f in range(KT)]
    gt = [st(sb, [128, TLOC], BF16, f"g{m}") for m in range(DFF // 128)]
    ones_b = st(sb, [128, 1], BF16, "ones_b")
    nc.vector.memset(ones_b[:], 1.0)
    maska = st(sb, [128, 2 * 256], BF16, "maska")
    nc.sync.dma_start(out=maska[:].rearrange("p (j q) -> p j q", j=2),
                      in_=P["maska"][:].transpose([1, 0, 2]))
    maskb = st(sb, [128, 512], BF16, "maskb")
    nc.sync.dma_start(out=maskb[:], in_=P["maskb"][:])

    def psx():
        return [st(ps, [128, TLOC], F32, t, bufs=b)
                for (t, b) in (("ps_x", 2), ("ps_x", 2),
                               ("ps_o", 2), ("ps_s", 3))]

    def load_wg(pname, g, k0=0, nk=KT):
        wt = st(wp, [128, nk * 512], BF16, "w", bufs=3)
        dma(wt[:], P[pname][g][:, 512 * k0:512 * (k0 + nk)])
        return wt

    def wproj(pname, mg, nk, rhs, pss):
        for k0 in range(0, nk, KT):
            wt = load_wg(pname, mg, k0, KT)
            for kk in range(KT):
                k = k0 + kk
                for m in range(4):
                    c0 = 512 * kk + 128 * m
                    nc.tensor.matmul(pss[m][:], wt[:, c0:c0 + 128],
                                     rhs[k][:],
                                     start=(k == 0), stop=(k == nk - 1))

    def ln_layer(scol, bcol, dst):
        s_ps = st(ps, [1, TLOC], F32, "ps_s", bufs=3)
        s2_ps = st(ps, [1, TLOC], F32, "ps_s", bufs=3)
        for f in range(KT):
            xb = st(pp, [128, TLOC], BF16, "xb", bufs=2)
            xsq = st(pp, [128, TLOC], BF16, "xsq", bufs=2)
            nc.gpsimd.tensor_copy(xb[:], x[f][:])
            nc.gpsimd.tensor_mul(xsq[:], x[f][:], x[f][:])
            nc.tensor.matmul(s_ps[:], ones_b[:], xb[:],
                             start=(f == 0), stop=(f == KT - 1))
            nc.tensor.matmul(s2_ps[:], ones_b[:], xsq[:],
                             start=(f == 0), stop=(f == KT - 1))
        mean = st(pp, [1, TLOC], F32, "stat", bufs=8)
        msq = st(pp, [1, TLOC], F32, "stat", bufs=8)
        mm = st(pp, [1, TLOC], F32, "stat", bufs=8)
        var = st(pp, [1, TLOC], F32, "stat", bufs=8)
        rinv = st(pp, [1, TLOC], F32, "stat", bufs=8)
        rstd = st(pp, [1, TLOC], F32, "stat", bufs=8)
        nc.vector.tensor_scalar_mul(mean[:], s_ps[:], 1.0 / D)
        nc.vector.tensor_scalar_mul(msq[:], s2_ps[:], 1.0 / D)
        nc.vector.tensor_mul(mm[:], mean[:], mean[:])
        nc.vector.scalar_tensor_tensor(var[:], msq[:], float(EPS), mm[:],
                                       ALU.add, ALU.subtract)
        nc.vector.reciprocal(rinv[:], var[:])
        nc.scalar.sqrt(rstd[:], rinv[:])
        m_bc = st(pp, [128, TLOC], F32, "mbc", bufs=2)
        r_bc = st(pp, [128, TLOC], F32, "rbc", bufs=2)
        nc.gpsimd.partition_broadcast(m_bc[:], mean[:])
        nc.gpsimd.partition_broadcast(r_bc[:], rstd[:])
        for f in range(KT):
            t0 = st(pp, [128, TLOC], F32, "lnt0", bufs=2)
            t1 = st(pp, [128, TLOC], F32, "lnt1", bufs=2)
            nc.vector.tensor_sub(t0[:], x[f][:], m_bc[:])
            nc.vector.tensor_mul(t1[:], t0[:], r_bc[:])
            nc.gpsimd.tensor_scalar(dst[f][:], t1[:], scol[f], bcol[f],
                                    ALU.mult, ALU.add)

    for f in range(KT):
        nc.sync.dma_start(out=x[f][:], in_=P["x0"][f])

    group = [list(range(G)), list(range(G, 2 * G))]

    for l in range(NL):
        lnp = st(pp, [128, 32], F32, "lnp", bufs=2)
        nc.sync.dma_start(out=lnp[:], in_=P[f"ln{l}"][:])
        bqkv = st(pp, [128, 24], F32, "bqkvt", bufs=2)
        nc.sync.dma_start(out=bqkv[:], in_=P[f"bqkv{l}"][:])
        bvrow = st(pp, [1, D], F32, "bvrow", bufs=2)
        nc.sync.dma_start(out=bvrow[:], in_=P[f"bvrow{l}"][:])
        bo = st(pp, [128, 8], F32, "bot", bufs=2)
        nc.sync.dma_start(out=bo[:], in_=P[f"bo{l}"][:])
        b1 = st(pp, [128, 32], F32, "b1t", bufs=2)
        nc.sync.dma_start(out=b1[:], in_=P[f"b1{l}"][:])
        b2 = st(pp, [128, 8], F32, "b2t", bufs=2)
        nc.sync.dma_start(out=b2[:], in_=P[f"b2{l}"][:])

        # LN1 -> xh
        ln_layer([lnp[:, f:f + 1] for f in range(KT)],
                 [lnp[:, 8 + f:9 + f] for f in range(KT)], xh)

        # AllGather the LN output x-hat within the group; K and V are then
        # computed (redundantly) for all 1024 tokens on every core, which
        # halves the collective bytes and removes the KV staging round-trip.
        xh_in = st(dram, [D, TLOC], BF16, "xh_in", bufs=2)
        xh_out = st(dram, [G * D, TLOC], BF16, "xh_out", bufs=2)
        for f in range(KT):
            nc.gpsimd.dma_start(out=xh_in[128 * f:128 * (f + 1), :],
                                in_=xh[f][:])
        if NOAG:
            for gg in range(G):
                nc.gpsimd.dma_start(
                    out=xh_out[D * gg:D * (gg + 1), :], in_=xh_in[:])
        else:
            nc.gpsimd.collective_compute(
                "AllGather", ALU.bypass, replica_groups=group,
                ins=[xh_in[:].opt()], outs=[xh_out[:].opt()])
        for f in range(KT):
            gsrc = xh_out[:].rearrange("(g r p) t -> r p g t", g=G, p=128)[f]
            nc.sync.dma_start(
                out=xhg[f][:].rearrange("p (g t) -> p g t", g=G), in_=gsrc)

        # Q projection (local tokens, feature-major)
        for mg in range(2):
            pss = psx()
            wproj(f"wqkv{l}", mg, KT, xh, pss)
            for m in range(4):
                fm = 4 * mg + m
                nc.vector.tensor_scalar_add(
                    q_sb[fm][:], pss[m][:], bqkv[:, fm:fm + 1])

        # K projection (all gathered tokens, feature-major -> ksb)
        for mg in range(2):
            wt = load_wg(f"wqkv{l}", 2 + mg)
            for m in range(4):
                fm = 4 * mg + m
                for nh in range(2):
                    psk = st(ps, [128, 512], F32, "ps_x", bufs=2)
                    for k in range(KT):
                        nc.tensor.matmul(
                            psk[:],
                            wt[:, 512 * k + 128 * m:512 * k + 128 * (m + 1)],
                            xhg[k][:, 512 * nh:512 * (nh + 1)],
                            start=(k == 0), stop=(k == KT - 1))
                    nc.vector.tensor_scalar_add(
                        ksb[fm][:, 512 * nh:512 * (nh + 1)], psk[:],
                        bqkv[:, 8 + fm:9 + fm])

        # V projection (all gathered tokens, token-major -> vsb scatter)
        bv_bc = st(pp, [128, D], F32, "bvbc", bufs=1)
        nc.gpsimd.partition_broadcast(bv_bc[:], bvrow[:])
        for j in range(8):
            nc.vector.memset(
                vsb[j][:].rearrange("p (h e) -> p h e", h=16)[:, :, 64:65],
                1.0)
        for n in range(2):
            wt = load_wg(f"wqkv{l}", 4 + n)
            for j in range(8):
                psv = st(ps, [128, 512], F32, "ps_s", bufs=3)
                for k in range(KT):
                    nc.tensor.matmul(
                        psv[:],
                        xhg[k][:, 128 * j:128 * (j + 1)],
                        wt[:, 512 * k:512 * (k + 1)],
                        start=(k == 0), stop=(k == KT - 1))
                dstv = vsb[j][:].rearrange(
                    "p (h e) -> p h e", h=16)[:, 8 * n:8 * (n + 1), 0:64]
                nc.vector.tensor_add(
                    dstv, psv[:].rearrange("p (h e) -> p h e", e=64),
                    bv_bc[:, 512 * n:512 * (n + 1)].rearrange(
                        "p (h e) -> p h e", e=64))

        # attention
        for h in range(H):
            f, po = h // 2, 64 * (h % 2)
            o_ps = st(ps, [65, TLOC], F32, "ps_o", bufs=2)
            # A blocks: full q width; mask applies only to q cols 0:128
            for jp in range(2):
                s_ps = st(ps, [128, 512], F32, "ps_s", bufs=3)
                for a in range(2):
                    j = A_BLOCKS[2 * jp + a]
                    nc.tensor.matmul(
                        s_ps[:, 256 * a:256 * (a + 1)],
                        ksb[f][po:po + 64, 128 * j:128 * (j + 1)],
                        q_sb[f][po:po + 64, :],
                        start=True, stop=True)
                pt = st(pp, [128, 512], BF16, "pt", bufs=4)
                nc.scalar.activation(pt[:], s_ps[:], AF.Exp,
                                     scale=float(1.0 / np.sqrt(DH)))
                ptv = pt[:].rearrange("p (a b q) -> p a b q", a=2, b=2)
                nc.vector.tensor_mul(
                    ptv[:, :, 0, :], ptv[:, :, 0, :],
                    maska[:].rearrange("p (j q) -> p j q", j=4)[
                        :, 2 * jp:2 * jp + 2])
                for a in range(2):
                    j = A_BLOCKS[2 * jp + a]
                    nc.tensor.matmul(
                        o_ps[:],
                        vsb[j][:, 65 * h:65 * (h + 1)],
                        pt[:, 256 * a:256 * (a + 1)],
                        start=(jp == 0 and a == 0), stop=False,
                        skip_group_check=True)
            # B blocks: only q cols 128:256; all four masked
            s_ps = st(ps, [128, 512], F32, "ps_s", bufs=3)
            for b in range(4):
                j = B_BLOCKS[b]
                nc.tensor.matmul(
                    s_ps[:, 128 * b:128 * (b + 1)],
                    ksb[f][po:po + 64, 128 * j:128 * (j + 1)],
                    q_sb[f][po:po + 64, 128:256],
                    start=True, stop=True)
            pt = st(pp, [128, 512], BF16, "pt", bufs=4)
            nc.scalar.activation(pt[:], s_ps[:], AF.Exp,
                                 scale=float(1.0 / np.sqrt(DH)))
            nc.vector.tensor_mul(pt[:], pt[:], maskb[:])
            for b in range(4):
                j = B_BLOCKS[b]
                nc.tensor.matmul(
                    o_ps[:, 128:256],
                    vsb[j][:, 65 * h:65 * (h + 1)],
                    pt[:, 128 * b:128 * (b + 1)],
                    start=False, stop=(b == 3), skip_group_check=True)
            dinv = st(pp, [1, TLOC], F32, "dinv", bufs=3)
            nc.vector.reciprocal(dinv[:], o_ps[64:65, :])
            dbc = st(pp, [64, TLOC], F32, "dbc", bufs=3)
            nc.gpsimd.partition_broadcast(dbc[:], dinv[:])
            nc.vector.tensor_mul(y_sb[f][po:po + 64, :], o_ps[0:64, :],
                                 dbc[:])

        # Wo + residual
        for mg in range(2):
            pss = psx()
            wproj(f"wo{l}", mg, KT, y_sb, pss)
            for m in range(4):
                fm = 4 * mg + m
                nc.vector.scalar_tensor_tensor(
                    x[fm][:], pss[m][:], bo[:, fm:fm + 1], x[fm][:],
                    ALU.add, ALU.add)

        # LN2 -> xh
        ln_layer([lnp[:, 16 + f:17 + f] for f in range(KT)],
                 [lnp[:, 24 + f:25 + f] for f in range(KT)], xh)

        # FFN
        for mg in range(8):
            pss = psx()
            wproj(f"w1{l}", mg, KT, xh, pss)
            for m in range(4):
                fm = 4 * mg + m
                nc.scalar.activation(gt[fm][:], pss[m][:], AF.Gelu,
                                     bias=b1[:, fm:fm + 1])
        for mg in range(2):
            pss = psx()
            wproj(f"w2{l}", mg, DFF // 128, gt, pss)
            for m in range(4):
                fm = 4 * mg + m
                nc.vector.scalar_tensor_tensor(
                    x[fm][:], pss[m][:], b2[:, fm:fm + 1], x[fm][:],
                    ALU.add, ALU.add)

    # final LN + LM head
    lnf = st(pp, [128, 16], F32, "lnft", bufs=1)
    nc.sync.dma_start(out=lnf[:], in_=P["lnf"][:])
    ln_layer([lnf[:, f:f + 1] for f in range(KT)],
             [lnf[:, 8 + f:9 + f] for f in range(KT)], xh)
    bl_sb = st(sb, [128, VPAD // 128], F32, "bl")
    nc.sync.dma_start(out=bl_sb[:], in_=P["bl"][:])
    for mg in range(VPAD // 512):
        pss = psx()
        wproj("wl", mg, KT, xh, pss)
        lsb = st(pp, [128, 1024], F32, "lout", bufs=2)
        for m in range(4):
            vm = 4 * mg + m
            nc.vector.tensor_scalar_add(lsb[:, 256 * m:256 * (m + 1)],
                                        pss[m][:], bl_sb[:, vm:vm + 1])
        dma(P["logits"][mg], lsb[:])

    ctx.close()


# ---------------------------------------------------------------------------
# host side
# ---------------------------------------------------------------------------

_CACHE = {}


def _tile_w(w):
    """(Din, Dout) -> (Dout/512, 128, (Din/128)*512) bf16 contiguous.

    Group g, partition p, column 512k+c = w[128k+p, 512g+c].
    """
    din, dout = w.shape
    t = w.reshape(din // 128, 128, dout // 512, 512).transpose(2, 1, 0, 3)
    return np.ascontiguousarray(t.reshape(dout // 512, 128,
                                          (din // 128) * 512)).astype(bfdt)


def _feat_cols(v):
    return np.ascontiguousarray(np.asarray(v, np.float32).reshape(-1, 128).T)


def _core_rows(c):
    a, b = _q_starts(c)
    return list(range(a, a + 128)) + list(range(b, b + 128))


def _core_masks(c):
    """maska (2, 128, 256), maskb (128, 512) for rank c."""
    qa = 128 * c + np.arange(128)
    qb = 1024 - 128 * (c + 1) + np.arange(128)
    ma = np.zeros((2, 128, 2, 128), np.float32)
    for jp in range(2):
        for a in range(2):
            kvrows = _kv_start(A_BLOCKS[2 * jp + a]) + np.arange(128)
            ma[jp, :, a] = (kvrows[:, None] <= qa[None, :])
    mb = np.zeros((128, 4, 128), np.float32)
    for b in range(4):
        kvrows = _kv_start(B_BLOCKS[b]) + np.arange(128)
        mb[:, b] = (kvrows[:, None] <= qb[None, :])
    return (ma.reshape(2, 128, 256).astype(bfdt),
            mb.reshape(128, 512).astype(bfdt))


def _get_nc():
    if "nc" not in _CACHE:
        _CACHE["nc"] = build_nc()
    return _CACHE["nc"]


def make_in_maps(tokens, emb, pos_emb, ln1_s, ln1_b, Wq, bq, Wk, bk, Wv, bv,
                 Wo, bo, ln2_s, ln2_b, W1, b1, W2, b2, lnf_s, lnf_b, Wl, bl):
    f32 = lambda a: np.asarray(a, np.float32)
    tokens = np.asarray(tokens)
    x0 = f32(emb)[tokens] + f32(pos_emb)[:T][None]      # (B, T, D)

    shared = {}
    for l in range(NL):
        wqkv = np.concatenate([f32(Wq)[l], f32(Wk)[l], f32(Wv)[l]], axis=1)
        shared[f"wqkv{l}"] = _tile_w(wqkv)
        shared[f"wo{l}"] = _tile_w(f32(Wo)[l])
        shared[f"w1{l}"] = _tile_w(f32(W1)[l])
        shared[f"w2{l}"] = _tile_w(f32(W2)[l])
        shared[f"ln{l}"] = np.concatenate(
            [_feat_cols(ln1_s[l]), _feat_cols(ln1_b[l]),
             _feat_cols(ln2_s[l]), _feat_cols(ln2_b[l])], axis=1)
        shared[f"bqkv{l}"] = np.concatenate(
            [_feat_cols(bq[l]), _feat_cols(bk[l]), _feat_cols(bv[l])], axis=1)
        shared[f"bvrow{l}"] = f32(bv)[l].reshape(1, D).copy()
        shared[f"bo{l}"] = _feat_cols(bo[l])
        shared[f"b1{l}"] = _feat_cols(b1[l])
        shared[f"b2{l}"] = _feat_cols(b2[l])
    shared["lnf"] = np.concatenate(
        [_feat_cols(lnf_s), _feat_cols(lnf_b)], axis=1)
    wl_pad = np.zeros((D, VPAD), np.float32)
    wl_pad[:, :V] = f32(Wl)
    shared["wl"] = _tile_w(wl_pad)
    bl_pad = np.zeros(VPAD, np.float32)
    bl_pad[:V] = f32(bl)
    shared["bl"] = _feat_cols(bl_pad)

    in_maps = []
    for core in range(NCORE):
        g, c = core // G, core % G
        m = dict(shared)
        xt = x0[g][_core_rows(c)].T                     # (D, 256)
        m["x0"] = np.ascontiguousarray(xt).reshape(KT, 128, TLOC).copy()
        ma, mb = _core_masks(c)
        m["maska"] = ma
        m["maskb"] = mb
        in_maps.append(m)
    return in_maps


def assemble(results):
    out = np.empty((B, T, V), np.float32)
    for core in range(NCORE):
        g, c = core // G, core % G
        lg = results[core]["logits"]                    # (63, 128, 1024)
        lg = lg.reshape(VPAD // 512, 128, 4, 256).transpose(0, 2, 1, 3)
        lg = lg.reshape(VPAD, TLOC)[:V]
        out[g, _core_rows(c), :] = lg.T
    return out


def kernel(**inputs):
    in_maps = make_in_maps(**inputs)
    res = run_bass_kernel_spmd(_get_nc(), in_maps, list(range(NCORE)))
    global LAST_EXEC_NS
    LAST_EXEC_NS = getattr(res, "exec_time_ns", None)
    return assemble(res.results)


LAST_EXEC_NS = None
